# revision 63
# baseline (speedup 1.0000x reference)
"""DeltaHebbianBlock Trainium2 kernel (v4, 254.5us vs 730.8us baseline).

Sharding: 8 cores = (B=2) x (H=4). Each core runs its head's delta-rule
chunked scan (C=128, degree-3 UT chain, rel_err ~1.1e-3 vs the 2e-2 gate)
and the partial output projection partial_bh = (alpha_h*o_bh) @ Wr_h^T.
Host gathers: out[b] = x[b] + sum_h partial (partial stored bf16).

Fully fused slot schedule per quarter (QT=1024, 8 chunks of C=128): slot n
emits P2-grams(q,n), P3-scan(q,n-3), P2-chain-tail one slot late (so the
scan's chain-critical PSUM drains lead the DVE/Act queues), P4 out-proj
(lag 8), and P1 v-proj/rk for q+1.  Key tricks:
- fp8e4m3 + DoubleRow matmuls for both DxD projections (K=256/instr,
  0.5 cyc/row); alpha folded into the oT drain scale, NOT the fp8 wrt
  (subnormal flush).
- wkcT stored negated and vcp/identity matmuls folded into the vnp PSUM
  group -> vnew accumulates fully in PSUM (no separate TSP).
- S decay folded into the sup matmul group via a gcv*I constant; S drain
  is a plain copy split jv-wise across Act/DVE (two interleaved chains).
- scan emits oT directly via transposed matmuls (no output transposes);
  (I+A0)(I+A0^2) chain with identity folds (no G0/Gh0 adds).
- PSUM: 8 banks exactly; one matmul group per bank at a time.
- GPSIMD/Pool cannot touch PSUM; it handles SBUF-only scalings.
"""
import os
import numpy as np
import ml_dtypes
from contextlib import ExitStack

import concourse.bass as bass
import concourse.mybir as mybir
import concourse.tile as tile
from concourse import bacc, bass_utils

B, T, D = 2, 8192, 1024
H, d, C = 4, 256, 128
NQ = 8                # quarter passes
QT = T // NQ          # 1024 tokens per pass
NCH = QT // C         # 8 chunks per pass

F32 = mybir.dt.float32
BF16 = mybir.dt.bfloat16
F8 = mybir.dt.float8e4
MULT = mybir.AluOpType.mult
ADD = mybir.AluOpType.add
ACT_COPY = None  # set in _build


def _build():
    nc = bacc.Bacc("TRN2", target_bir_lowering=False, debug=False,
                   num_devices=int(os.environ.get("K_NCORES", "8")))
    xT_d = nc.dram_tensor("xt", (D, T), F8, kind="ExternalInput")
    xh_d = nc.dram_tensor("xh", (T, d), BF16, kind="ExternalInput")
    wwt_d = nc.dram_tensor("wwt", (D, d), F8, kind="ExternalInput")
    wrt_d = nc.dram_tensor("wrt", (d, D), F8, kind="ExternalInput")
    alsc_d = nc.dram_tensor("alsc", (128, 1), F32, kind="ExternalInput")
    mb_d = nc.dram_tensor("mb", (C, C), F32, kind="ExternalInput")
    mc_d = nc.dram_tensor("mc", (C, C), F32, kind="ExternalInput")
    mit_d = nc.dram_tensor("mit", (C, C), F32, kind="ExternalInput")
    id_d = nc.dram_tensor("ident", (128, 128), BF16, kind="ExternalInput")
    gcvid_d = nc.dram_tensor("gcvid", (128, 128), BF16, kind="ExternalInput")
    gpb_d = nc.dram_tensor("gpbf", (128, QT), BF16, kind="ExternalInput")
    gpt_d = nc.dram_tensor("gpt", (128, 1), F32, kind="ExternalInput")
    part_d = nc.dram_tensor("partial", (T, D), BF16, kind="ExternalOutput")

    COPY = mybir.ActivationFunctionType.Copy
    SQRT = mybir.ActivationFunctionType.Sqrt
    GP = nc.gpsimd if os.environ.get("K_POOL", "1") == "1" else nc.vector

    with ExitStack() as ctx:
        tc = ctx.enter_context(tile.TileContext(nc))
        consts = ctx.enter_context(tc.tile_pool(name="consts", bufs=1))
        qx = ctx.enter_context(tc.tile_pool(name="qx", bufs=2))
        qa = ctx.enter_context(tc.tile_pool(name="qa", bufs=2))
        chp = ctx.enter_context(tc.tile_pool(name="chp", bufs=3))
        vnw = ctx.enter_context(tc.tile_pool(name="vnw", bufs=4))
        st_p = ctx.enter_context(tc.tile_pool(name="stp", bufs=2))
        scr = ctx.enter_context(tc.tile_pool(name="scr", bufs=2))
        ps = ctx.enter_context(tc.tile_pool(name="ps", bufs=1, space="PSUM"))

        # ---- constants / weights in SBUF ----
        wwt_s = consts.tile([128, 8, d], F8)
        nc.sync.dma_start(wwt_s[:], wwt_d.ap().rearrange("(kb p) j -> p kb j", p=128))
        wrt_s = consts.tile([128, 2, D], F8)
        nc.sync.dma_start(wrt_s[:], wrt_d.ap().rearrange("(kt p) n -> p kt n", p=128))
        mb2_s = consts.tile([128, 2, 128], F32)
        mc2_s = consts.tile([128, 2, 128], F32)
        mit2_s = consts.tile([128, 2, 128], F32)
        id2_s = consts.tile([128, 2, 128], BF16)
        for ch in range(2):
            nc.sync.dma_start(mb2_s[:, ch, :], mb_d.ap())
            nc.sync.dma_start(mc2_s[:, ch, :], mc_d.ap())
            nc.sync.dma_start(mit2_s[:, ch, :], mit_d.ap())
            nc.sync.dma_start(id2_s[:, ch, :], id_d.ap())
        id_s = consts.tile([128, 128], BF16)
        nc.sync.dma_start(id_s[:], id_d.ap())
        gcvid_s = consts.tile([128, 128], BF16)
        nc.sync.dma_start(gcvid_s[:], gcvid_d.ap())
        gpb_s = consts.tile([128, QT], BF16)
        nc.sync.dma_start(gpb_s[:], gpb_d.ap())
        gpt_s = consts.tile([128, 1], F32)
        nc.sync.dma_start(gpt_s[:], gpt_d.ap())
        alsc_s = consts.tile([128, 1], F32)
        nc.sync.dma_start(alsc_s[:], alsc_d.ap())
        ones_s = consts.tile([128, 1], BF16)
        nc.gpsimd.memset(ones_s[:], 1.0)

        S_bf = consts.tile([128, 2, d], BF16)
        nc.gpsimd.memset(S_bf[:], 0.0)

        QS = {}  # per-quarter tile sets

        def qtiles(qq):
            if qq in QS:
                return QS[qq]
            t = {}
            t["xT"] = qx.tile([128, 8, QT], F8, tag="xT", name="xT")
            t["xh"] = qa.tile([128, 8, d], BF16, tag="xh", name="xh")
            t["v_nat"] = qa.tile([128, 8, d], BF16, tag="v_nat", name="v_nat")
            t["nrm2"] = scr.tile([128, 8], F32, tag="nrm2", name="nrm2", bufs=3)
            t["nrm"] = scr.tile([128, 8], F32, tag="nrm", name="nrm", bufs=3)
            t["inv"] = scr.tile([128, 8], F32, tag="inv", name="inv", bufs=3)
            t["rk"] = qa.tile([128, 8, d], BF16, tag="rk", name="rk")
            t["wk"] = qa.tile([128, 8, d], BF16, tag="wk", name="wk")
            t["wkgN"] = qa.tile([128, 8, d], BF16, tag="wkgN", name="wkgN")
            t["rkT"] = qa.tile([128, 2, QT + 1], BF16, tag="rkT", name="rkT")
            t["rkgT"] = qa.tile([128, 2, QT], BF16, tag="rkgT", name="rkgT")
            t["wkcT"] = qa.tile([128, 2, QT], BF16, tag="wkcT", name="wkcT")
            t["inT"] = qa.tile([128, NCH, C], BF16, tag="inT", name="inT")
            t["AT"] = qa.tile([128, NCH, C], BF16, tag="AT", name="AT")
            t["oT"] = qa.tile([128, 2, QT], F8, tag="oT", name="oT")
            QS[qq] = t
            return t

        def loads(qq):
            t = qtiles(qq)
            qt0 = qq * QT
            nc.sync.dma_start(
                t["xT"][:], xT_d.ap()[:, qt0:qt0 + QT].rearrange(
                    "(kb p) t -> p kb t", p=128))
            nc.sync.dma_start(
                t["xh"][:], xh_d.ap()[qt0:qt0 + QT, :].rearrange(
                    "(tt p) j -> p tt j", p=128))


        # ---------- P1: v-proj + rk for token-tile n ----------
        def p1_slice(qq, n):
            t = qtiles(qq)
            vps = ps.tile([128, d], F32, tag="bigp", bufs=2, name="vps")
            for kp in range(4):
                nc.tensor.matmul(vps[:], t["xT"][:, 2 * kp:2 * kp + 2, n * 128:(n + 1) * 128],
                                 wwt_s[:, 2 * kp:2 * kp + 2, :], start=(kp == 0), stop=(kp == 3),
                                 perf_mode=mybir.MatmulPerfMode.DoubleRow)
            nc.scalar.activation(t["v_nat"][:, n, :], vps[:], COPY)
            sq = scr.tile([128, d], F32, tag="sq", name="sq")
            nc.scalar.activation(sq[:], t["xh"][:, n, :],
                                 mybir.ActivationFunctionType.Square,
                                 accum_out=t["nrm2"][:, n:n + 1])
            nc.scalar.activation(t["nrm"][:, n:n + 1], t["nrm2"][:, n:n + 1], SQRT)
            nc.vector.reciprocal(t["inv"][:, n:n + 1], t["nrm"][:, n:n + 1])
            nc.gpsimd.tensor_scalar(t["rk"][:, n, :], t["xh"][:, n, :],
                                    t["inv"][:, n:n + 1], None, MULT)
            tp = ps.tile([128, 2, 128], BF16, tag="cgtp", bufs=1, name="tp")
            for kt in range(2):
                nc.tensor.transpose(tp[:, kt, :],
                                    t["rk"][:, n, kt * 128:(kt + 1) * 128], id_s[:])
            nc.vector.tensor_copy(
                t["rkT"][:, :, 1 + n * 128:1 + (n + 1) * 128], tp[:])

        # ---------- dprep: shift/scale prep for quarter qq ----------
        def dprep(qq):
            t = qtiles(qq)
            if qq == 0:
                nc.gpsimd.memset(t["rkT"][:, :, 0:1], 0.0)
                nc.gpsimd.memset(t["wk"][0:1, 0:1, :], 0.0)
            else:
                tprev = QS[qq - 1]
                nc.vector.tensor_copy(t["rkT"][:, :, 0:1],
                                      tprev["rkT"][:, :, QT:QT + 1])
                nc.sync.dma_start(t["wk"][0:1, 0:1, :],
                                  tprev["rk"][127:128, 7:8, :])
            nc.sync.dma_start(t["wk"][1:128, :, :], t["rk"][0:127, :, :])
            nc.sync.dma_start(t["wk"][0:1, 1:8, :], t["rk"][127:128, 0:7, :])
            nc.gpsimd.tensor_scalar(t["wkgN"][:], t["wk"][:], gpt_s[:, 0:1],
                                    None, MULT)
            for kt in range(2):
                nc.gpsimd.tensor_mul(t["rkgT"][:, kt, :],
                                     t["rkT"][:, kt, 1:QT + 1], gpb_s[:])

        # ---------- P2: chain, pair-batched (called per slot) ----------
        # pair state carried between even/odd slots
        pair = {}

        def p2_slot(qq, n):
            t = qtiles(qq)
            half = n % 2
            w0 = n * C
            if half == 0:
                pair["g"] = ps.tile([128, 2, 2, 128], F32, tag="g", bufs=2,
                                    name="gpair")
            g = pair["g"]
            for kt in range(2):
                nc.tensor.matmul(g[:, half, 0, :], t["rkT"][:, kt, w0:w0 + 128],
                                 t["rkT"][:, kt, w0:w0 + 128],
                                 start=(kt == 0), stop=(kt == 1))
            for kt in range(2):
                nc.tensor.matmul(g[:, half, 1, :], t["rkT"][:, kt, w0:w0 + 128],
                                 t["rkT"][:, kt, w0 + 1:w0 + 129],
                                 start=(kt == 0), stop=(kt == 1))
            if half == 0:
                return
            # odd slot: drains + chain for the pair (chunks n-1, n)
            p0 = n - 1
            B0 = chp.tile([128, 2, 128], BF16, tag="B0", name="B0")
            nc.vector.tensor_mul(B0[:], g[:, :, 0, :], mb2_s[:])
            C0 = chp.tile([128, 2, 128], BF16, tag="C0", name="C0")
            nc.vector.tensor_mul(C0[:], g[:, :, 0, :], mc2_s[:])
            nc.vector.tensor_mul(t["inT"][:, p0:p0 + 2, :], g[:, :, 1, :],
                                 mit2_s[:])
            c1p = ps.tile([128, 2, 128], F32, tag="cgtp", bufs=1, name="c1p")
            for ch in range(2):
                nc.tensor.matmul(c1p[:, ch, :], B0[:, ch, :], C0[:, ch, :])
            C1 = chp.tile([128, 2, 128], BF16, tag="C1", name="C1")
            nc.scalar.activation(C1[:], c1p[:], COPY)
            # g1p = (I + B0)^T C1 = C1 + C0 C1 ; AT' = g1p + C0 = A^T - I
            g1p = ps.tile([128, 2, 128], F32, tag="cgtp", bufs=1, name="g1p")
            for ch in range(2):
                nc.tensor.matmul(g1p[:, ch, :], id_s[:], C1[:, ch, :],
                                 start=True, stop=False)
                nc.tensor.matmul(g1p[:, ch, :], B0[:, ch, :], C1[:, ch, :],
                                 start=False, stop=True)
            nc.vector.tensor_add(t["AT"][:, p0:p0 + 2, :], g1p[:], C0[:])
            wcp = ps.tile([128, 2, 2, 128], F32, tag="g", bufs=2, name="wcp")
            for ch in range(2):
                for jb in range(2):
                    nc.tensor.matmul(
                        wcp[:, ch, jb, :],
                        t["wk"][:, p0 + ch, jb * 128:(jb + 1) * 128],
                        t["AT"][:, p0 + ch, :])
            # negated store: wkcT = -(A wk)^T = -(wcp + wk^T)
            for ch in range(2):
                c0 = w0 - C + ch * 128
                nc.vector.scalar_tensor_tensor(
                    t["wkcT"][:, :, c0:c0 + 128], wcp[:, ch, :, :], -1.0,
                    t["rkT"][:, :, c0:c0 + 128], MULT, mybir.AluOpType.subtract)

        # ---------- P3: scan chunk (jv-split chains) ----------
        def p3_chunk(qq, n):
            t = qtiles(qq)
            w0 = n * C
            vnp = ps.tile([128, 2, 128], F32, tag="scan", bufs=3, name="vnp")
            nc.tensor.matmul(vnp[:, :, :], id_s[:], t["v_nat"][:, n, :],
                             start=True, stop=False)
            nc.tensor.matmul(vnp[:, :, :], t["AT"][:, n, :], t["v_nat"][:, n, :],
                             start=False, stop=False)
            for jv in range(2):
                for jb in range(2):
                    nc.tensor.matmul(vnp[:, jv, :], t["wkcT"][:, jb, w0:w0 + 128],
                                     S_bf[:, jb, jv * 128:(jv + 1) * 128],
                                     start=False, stop=(jv == 1 and jb == 1))
            vnew = vnw.tile([128, d], BF16, tag="vnew", name="vnew")
            nc.scalar.activation(vnew[:, 0:128], vnp[:, 0, :], COPY)
            nc.vector.tensor_copy(vnew[:, 128:256], vnp[:, 1, :])
            ot = ps.tile([128, 2, 128], F32, tag="scan", bufs=3, name="ot")
            for jv in range(2):
                for jb in range(2):
                    nc.tensor.matmul(ot[:, jv, :],
                                     S_bf[:, jb, jv * 128:(jv + 1) * 128],
                                     t["rkgT"][:, jb, w0:w0 + 128],
                                     start=(jb == 0), stop=False)
                nc.tensor.matmul(ot[:, jv, :],
                                 vnew[:, jv * 128:(jv + 1) * 128],
                                 t["inT"][:, n, :], start=False, stop=True)
            nc.scalar.activation(t["oT"][:, :, w0:w0 + 128], ot[:], COPY,
                                 scale=alsc_s[:, 0:1])
            sup = ps.tile([128, 2, d], F32, tag="scan", bufs=3, name="sup")
            for jb in range(2):
                nc.tensor.matmul(sup[:, jb, :], gcvid_s[:], S_bf[:, jb, :],
                                 start=True, stop=False)
                for jv in range(2):
                    nc.tensor.matmul(sup[:, jb, jv * 128:(jv + 1) * 128],
                                     t["wkgN"][:, n, jb * 128:(jb + 1) * 128],
                                     vnew[:, jv * 128:(jv + 1) * 128],
                                     start=False, stop=(jv == 1))
            # S <- sup (gcv*S folded into matmul group)
            nc.vector.tensor_copy(S_bf[:, :, 0:128], sup[:, :, 0:128])
            nc.scalar.activation(S_bf[:, :, 128:256], sup[:, :, 128:256], COPY)

        # ---------- P4: out-projection ----------
        def p4_chunk(qq, n, st):
            t = qtiles(qq)
            for nh in range(2):
                pps = ps.tile([128, 512], F32, tag="bigp", bufs=2, name="pps")
                nc.tensor.matmul(pps[:], t["oT"][:, :, n * 128:(n + 1) * 128],
                                 wrt_s[:, :, nh * 512:(nh + 1) * 512],
                                 start=True, stop=True,
                                 perf_mode=mybir.MatmulPerfMode.DoubleRow)
                if nh == 0:
                    nc.vector.tensor_copy(st[:, n % 2, 0:512], pps[:])
                else:
                    nc.scalar.activation(st[:, n % 2, 512:1024], pps[:], COPY)
            if n % 2 == 1:
                roff = qq * QT + (n - 1) * 128
                nc.sync.dma_start(
                    part_d.ap()[roff:roff + 256, :].rearrange(
                        "(c p) j -> p c j", p=128), st[:])

        # ---------------- schedule ----------------
        loads(0)
        loads(1)
        for n in range(NCH):
            p1_slice(0, n)
        dprep(0)
        LOADS_AHEAD = True
        # P4(q, m) runs at slot m+4 of quarter q (oT(q, m) drained at m+2);
        # chunks 6,7 spill to slots 0,1 of the next quarter.
        p4q = []   # pending (qq, chunk) in order
        st_box = [None]

        def p4_push(qq, m):
            p4q.append((qq, m))

        def p4_pop():
            if not p4q:
                return
            qq, m = p4q.pop(0)
            if m % 2 == 0:
                st_box[0] = st_p.tile([128, 2, QT], BF16, tag="st", name="st")
            p4_chunk(qq, m, st_box[0])

        for q in range(NQ):
            if 2 <= q + 1 < NQ:
                loads(q + 1)
            for n in range(10):
                if n < 8:
                    p2_slot(q, n)
                if 2 <= n:
                    p3_chunk(q, n - 2)
                    p4_push(q, n - 2)
                LAG = int(os.environ.get("K_P4LAG", "8"))
                while p4q and (p4q[0][0] < q or n - p4q[0][1] >= LAG):
                    p4_pop()
                    break
                if q < NQ - 1 and n < 8:
                    p1_slice(q + 1, n)
            if q < NQ - 1:
                dprep(q + 1)
            if q >= 1:
                QS.pop(q - 1, None)
        while p4q:
            p4_pop()
    nc.compile()
    return nc


_NC = None
LAST_EXEC_NS = None
LAST_TRACE = None


def _bf16(a):
    return np.ascontiguousarray(np.asarray(a).astype(ml_dtypes.bfloat16))


def _f8(a):
    return np.ascontiguousarray(np.asarray(a).astype(ml_dtypes.float8_e4m3))


def kernel(out, Ww, Wr, decay, log_alpha):
    global _NC
    out = np.asarray(out, dtype=np.float32)
    Ww = np.asarray(Ww, dtype=np.float32)
    Wr = np.asarray(Wr, dtype=np.float32)
    decay = np.asarray(decay, dtype=np.float32)
    log_alpha = np.asarray(log_alpha, dtype=np.float32)
    gamma = 1.0 / (1.0 + np.exp(-decay.astype(np.float64)))
    alpha = np.exp(log_alpha.astype(np.float64))

    if _NC is None:
        _NC = _build()
    nc = _NC

    pc = np.arange(C)
    xT_b = [_f8(out[b].T) for b in range(B)]
    in_maps = []
    for ci in range(8):
        b, h = ci // 4, ci % 4
        g = gamma[h]
        Ls = np.tril(g ** np.maximum(pc[:, None] - pc[None, :], 0), -1)
        mb = (-Ls).astype(np.float32)
        mit = np.triu(g ** np.maximum(pc[None, :] - pc[:, None], 0), 1).astype(np.float32)
        gp = (g ** (np.arange(QT) % C)).astype(np.float32)
        gpb = np.broadcast_to(gp[None, :], (128, QT))
        gpt = (g ** (C - 1 - np.arange(128)))[:, None].astype(np.float32)
        in_maps.append({
            "xt": xT_b[b],
            "xh": _bf16(out[b][:, h * d:(h + 1) * d]),
            "wwt": _f8(Ww[h * d:(h + 1) * d, :].T),
            "wrt": _f8(Wr[:, h * d:(h + 1) * d].T),
            "alsc": np.full((128, 1), alpha[h], np.float32),
            "mb": mb, "mc": np.ascontiguousarray(mb.T),
            "mit": mit,
            "ident": _bf16(np.eye(128, dtype=np.float32)),
            "gcvid": _bf16((g ** C) * np.eye(128, dtype=np.float32)),
            "gpbf": _bf16(gpb),
            "gpt": gpt,
        })

    ncore = int(os.environ.get("K_NCORES", "8"))
    res = bass_utils.run_bass_kernel_spmd(
        nc, in_maps[:ncore], core_ids=list(range(ncore)),
        trace=bool(os.environ.get("K_TRACE")))
    global LAST_EXEC_NS, LAST_TRACE
    LAST_EXEC_NS = res.exec_time_ns
    LAST_TRACE = res.instructions_and_trace
    final = out.copy()
    for ci in range(len(res.results)):
        b = ci // 4
        final[b] += res.results[ci]["partial"].astype(np.float32)
    return final
